# revision 6
# baseline (speedup 1.0000x reference)
"""OHNM (online hard negative mining) MSE loss on 8 Trainium2 NeuronCores.

Reference computation (per map, maps = character & affinity):
    all_loss = (pred - target)^2            # N = 64*512*512 pixels
    pos_sum  = sum(all_loss * weight        # over pixels with target != 0
    num_pos  = count(target != 0)
    topk     = top-1000 of all_loss over pixels with target == 0
    k        = min(1000, 4*num_pos, num_neg)
    loss     = (pos_sum + sum(topk[:k])) / (num_pos + k)
Result = loss_character + loss_affinity  (f32 scalar).

Sharding: data-parallel over batch, 8 batches per core. Each core computes,
per map, per batch-tile [128 x 2048]:
  - per-partition accumulated positive weighted loss  (tensor_tensor_reduce)
  - per-partition positive count                      (ACT Sign + accum)
  - top-8 negative losses per (partition, tile) chunk (DVE max8)
Host gathers the 8 cores' partials and does the exact final top-k reduce over
the candidate set. Candidate coverage is exact unless some 2048-element chunk
holds >8 of the global top-1000 (verified on host; falls back to exact numpy
in that astronomically unlikely case).
"""

import sys

sys.path.insert(0, "/opt/trn_rl_repo")

import numpy as np

import concourse.bacc as bacc
import concourse.tile as tile
from concourse import mybir
from concourse.bass_utils import run_bass_kernel_spmd

B, C, H, W = 64, 2, 512, 512
N_CORES = 8
BPC = B // N_CORES  # batches per core
P = 128
F = (H * W) // P  # 2048 elements per partition per batch-map
NT = BPC  # tiles per map per core
K_MAX = 1000
N_PIX = B * H * W

_CACHE = {}


def _build_nc():
    f32 = mybir.dt.float32
    bf16 = mybir.dt.bfloat16
    nc = bacc.Bacc()
    pred = nc.declare_dram_parameter("pred", [BPC, C, P, F], f32, isOutput=False)
    cmap = nc.declare_dram_parameter("cmap", [BPC, P, F], f32, isOutput=False)
    amap = nc.declare_dram_parameter("amap", [BPC, P, F], f32, isOutput=False)
    cw = nc.declare_dram_parameter("cw", [BPC, P, F], f32, isOutput=False)
    aw = nc.declare_dram_parameter("aw", [BPC, P, F], f32, isOutput=False)
    cand_o = nc.declare_dram_parameter("cand", [P, 2 * NT * 8], f32, isOutput=True)
    psum_o = nc.declare_dram_parameter("psums", [P, 2 * NT], f32, isOutput=True)
    cnt_o = nc.declare_dram_parameter("cnts", [P, 2 * NT], f32, isOutput=True)

    with tile.TileContext(nc) as tc:
        with (
            tc.tile_pool(name="io", bufs=3) as io,
            tc.tile_pool(name="work", bufs=2) as work,
            tc.tile_pool(name="scr", bufs=2) as scr,
            tc.tile_pool(name="singles", bufs=1) as singles,
        ):
            zeros = singles.tile([P, F], f32)
            nc.vector.memset(zeros, 0.0)
            candt = singles.tile([P, 2 * NT * 8], f32)
            post = singles.tile([P, 2 * NT], f32)
            cntt = singles.tile([P, 2 * NT], f32)

            for m, (tmap, wmap, ch) in enumerate(((cmap, cw, 0), (amap, aw, 1))):
                for bi in range(NT):
                    j = m * NT + bi
                    p_t = io.tile([P, F], f32, tag="p")
                    t_t = io.tile([P, F], f32, tag="t")
                    w_t = io.tile([P, F], f32, tag="w")
                    # t goes via SWDGE (gpsimd) so no compute instruction ever
                    # waits on two DMAHW semaphores (codegen allows at most one
                    # sync-wait per semaphore type per instruction).
                    nc.sync.dma_start(out=p_t, in_=pred[bi, ch])
                    nc.gpsimd.dma_start(out=t_t, in_=tmap[bi])
                    nc.sync.dma_start(out=w_t, in_=wmap[bi])

                    # d = pred - target
                    d = work.tile([P, F], f32, tag="d")
                    nc.gpsimd.tensor_sub(d, p_t, t_t)

                    # l = d^2 (all pixels); negv starts as the same
                    l = work.tile([P, F], f32, tag="l")
                    negv = work.tile([P, F], f32, tag="negv")
                    nc.scalar.square(l, d)
                    nc.scalar.square(negv, d)

                    # positive count: sum over free dim of sign(target)
                    junk2 = scr.tile([P, F], bf16, tag="junk2")
                    nc.scalar.activation(
                        out=junk2,
                        in_=t_t,
                        func=mybir.ActivationFunctionType.Sign,
                        accum_out=cntt[:, j : j + 1],
                    )

                    # negv: zero where target != 0 -> negative-only losses
                    # (mask must be integer dtype; f32 bits are nonzero iff
                    # the target is nonzero — targets are >= 0, never -0.0)
                    nc.vector.copy_predicated(negv, t_t.bitcast(mybir.dt.uint32), zeros)

                    # lp = l - negv: loss at positives, 0 at negatives
                    lp = work.tile([P, F], f32, tag="lp")
                    nc.vector.tensor_sub(lp, l, negv)

                    # positive weighted sum: wlp = lp * w on DVE, then the
                    # per-partition reduction rides ACT's activation accumulator
                    # (tensor_tensor_reduce is unsupported by this runtime)
                    wlp = work.tile([P, F], f32, tag="wlp")
                    nc.vector.tensor_mul(wlp, lp, w_t)
                    junk = scr.tile([P, F], bf16, tag="junk")
                    nc.scalar.activation(
                        out=junk,
                        in_=wlp,
                        func=mybir.ActivationFunctionType.Identity,
                        accum_out=post[:, j : j + 1],
                    )

                    # top-8 negative losses of this [128,2048] chunk per row
                    nc.vector.max(out=candt[:, j * 8 : (j + 1) * 8], in_=negv)

            nc.sync.dma_start(out=cand_o[:], in_=candt)
            nc.sync.dma_start(out=psum_o[:], in_=post)
            nc.sync.dma_start(out=cnt_o[:], in_=cntt)
    nc.compile()
    return nc


def _get_nc():
    if "nc" not in _CACHE:
        _CACHE["nc"] = _build_nc()
    return _CACHE["nc"]


def _ohnm_np(pred, target, weight):
    """Exact numpy fallback, mirrors the reference."""
    all_loss = (pred - target) ** 2
    pos_mask = target != 0
    num_pos = int(pos_mask.sum())
    num_neg = pred.size - num_pos
    pos_sum = float((all_loss * weight)[pos_mask].astype(np.float64).sum())
    neg_loss = np.where(pos_mask, -np.inf, all_loss)
    k = min(K_MAX, 4 * num_pos, num_neg)
    topk = np.sort(neg_loss.ravel())[-K_MAX:][::-1]
    neg_sum = float(topk[:k].astype(np.float64).sum())
    return np.float32((pos_sum + neg_sum) / np.float64(num_pos + k))


def _combine_map(results, m):
    """Host-side final reduce for one map from the 8 cores' partials."""
    pos_sum = 0.0
    num_pos = 0.0
    cands = []
    for r in results:
        pos_sum += float(r["psums"][:, m * NT : (m + 1) * NT].astype(np.float64).sum())
        num_pos += float(r["cnts"][:, m * NT : (m + 1) * NT].astype(np.float64).sum())
        cands.append(r["cand"][:, m * NT * 8 : (m + 1) * NT * 8].reshape(P, NT, 8))
    cand = np.stack(cands)  # [cores, P, NT, 8] descending within each chunk
    num_pos = int(round(num_pos))
    num_neg = N_PIX - num_pos
    k = min(K_MAX, 4 * num_pos, num_neg)
    flat = np.sort(cand.ravel())[::-1]
    neg_sum = float(flat[:k].astype(np.float64).sum()) if k > 0 else 0.0
    ok = True
    if k > 0:
        tau = flat[k - 1]
        # A chunk can only hide a missed top-k element if its own 8th-largest
        # (the smallest we kept) is strictly above the k-th candidate.
        chunk_min = cand[..., 7]
        ok = not bool((chunk_min > tau).any())
    loss = np.float32((pos_sum + neg_sum) / np.float64(num_pos + k))
    return loss, ok


def kernel(output, character_map, affinity_map, character_weight, affinity_weight):
    output = np.asarray(output, dtype=np.float32)
    character_map = np.asarray(character_map, dtype=np.float32)
    affinity_map = np.asarray(affinity_map, dtype=np.float32)
    character_weight = np.asarray(character_weight, dtype=np.float32)
    affinity_weight = np.asarray(affinity_weight, dtype=np.float32)

    nc = _get_nc()
    in_maps = []
    for i in range(N_CORES):
        sl = slice(i * BPC, (i + 1) * BPC)
        in_maps.append(
            {
                "pred": np.ascontiguousarray(output[sl]).reshape(BPC, C, P, F),
                "cmap": np.ascontiguousarray(character_map[sl]).reshape(BPC, P, F),
                "amap": np.ascontiguousarray(affinity_map[sl]).reshape(BPC, P, F),
                "cw": np.ascontiguousarray(character_weight[sl]).reshape(BPC, P, F),
                "aw": np.ascontiguousarray(affinity_weight[sl]).reshape(BPC, P, F),
            }
        )
    results = run_bass_kernel_spmd(nc, in_maps, list(range(N_CORES))).results

    loss_c, ok_c = _combine_map(results, 0)
    loss_a, ok_a = _combine_map(results, 1)
    if not ok_c:
        flat = output.transpose(0, 2, 3, 1).reshape(-1, C)
        loss_c = _ohnm_np(
            flat[:, 0], character_map.reshape(-1), character_weight.reshape(-1)
        )
    if not ok_a:
        flat = output.transpose(0, 2, 3, 1).reshape(-1, C)
        loss_a = _ohnm_np(
            flat[:, 1], affinity_map.reshape(-1), affinity_weight.reshape(-1)
        )
    return np.array(np.float32(loss_c) + np.float32(loss_a), dtype=np.float32)
